# revision 31
# baseline (speedup 1.0000x reference)
"""GAT 2-layer GNN (PyG GATConv semantics) on 8 Trainium2 NeuronCores.

Strategy: nodes row-partitioned across 8 cores. Phase 1 computes each
core's node records (xp | a_s, plus a_d kept in SBUF) from its own x
shard (row-major, transposed on device by DMA), then an AllGather
shares the record table. Edges are sorted by destination and grouped
into 128-node destination tiles x 128-edge chunks; per-edge source
records are fetched with dma_gather (int16 indices, lo/hi table split
for N>32768). Segment softmax + scatter-add are one-hot matmuls on the
tensor engine (edges on the contraction dim) accumulating
[denom | sum(ex*xp)] in PSUM. Layer-2 node scalars are all-gathered,
then layer 2 repeats the edge pass at H=C=1.

Per-core inputs are packed into two int16 blobs: blob A (x shard +
weights) is device_put before the edge tables are computed, so its
transfer overlaps host prep; blob B (edge tables) follows. Replicated
rows (one-hot helpers, W2) ship once and are expanded on device by
doubling DMAs. The compiled program + jitted executable are cached at
module level so repeat calls only pay host prep + transfer + execution.

Self-contained: only needs numpy + ml_dtypes + concourse (bass).
"""
import numpy as np
import ml_dtypes

import concourse.bacc as bacc
import concourse.mybir as mybir
import concourse.tile as tile

# ---- model constants (hardcoded for this problem) ----
F_IN = 128
H1, C1 = 8, 32
D1 = H1 * C1            # 256
RECW = 384              # record row: [xp 256 | a_s 8 | pad] bf16 -> 768B (%256)
L2W = 64                # layer-2 record row: [xp2 | pad] f32 -> 256B (%256)
NEG = 0.2
N_CORES = 8
P = 128
SPLIT_AT = 1 << 15      # int16 index split

F32 = mybir.dt.float32
BF16 = mybir.dt.bfloat16
I32 = mybir.dt.int32
I16 = mybir.dt.int16
I8 = mybir.dt.int8
AF = mybir.ActivationFunctionType

_NPD = {BF16: ml_dtypes.bfloat16, F32: np.float32, I16: np.int16, I8: np.int8}


def _split_sync_waits(nc, limit=1):
    """This container's walrus rejects >1 sem wait per instruction; move
    excess waits onto preceding same-engine EventSemaphore carriers."""
    import concourse.mybir as mb
    n_new = 0
    for fn in nc.m.functions:
        for blk in fn.blocks:
            out = []
            for inst in blk.instructions:
                si = inst.sync_info
                if si is not None and len(si.on_wait) > limit:
                    waits = list(si.on_wait)
                    extra, keep = waits[:-limit], waits[-limit:]
                    si.on_wait = keep
                    for j in range(0, len(extra), limit):
                        w = mb.InstEventSemaphore(
                            name=f"{inst.name}_w{j}", ins=[], outs=[]
                        )
                        w.engine = inst.engine
                        w.sync_info = mb.SyncInfo(
                            on_update=[], on_wait=extra[j : j + limit]
                        )
                        out.append(w)
                        n_new += 1
                out.append(inst)
            blk.instructions = out
    return n_new


def _mk_layout(sections):
    """name -> (offset_i16, shape, mybir dtype); offsets 256B-aligned so
    F32 bitcasts stay element-aligned."""
    lay, off = {}, 0
    for name, shp, dt_ in sections:
        n16 = int(np.prod(shp)) * mybir.dt.size(dt_) // 2
        lay[name] = (off, shp, dt_)
        off += (n16 + 127) & ~127
    return lay, off


def _layout_a(NPC, NT):
    # x ships int8 with a per-node f32 scale; rows padded to whole
    # 128-row tiles (pad rows quantize to 0 with scale 0)
    return _mk_layout([
        ("xq", (NT * P, F_IN), I8),
        ("xscale", (NT * P, 1), F32),
        ("W1ext", (F_IN, D1 + 2 * H1), BF16),
        ("iota1", (1, P), BF16),
        ("W2row", (1, D1), F32),
        ("iotac", (P, 1), F32),
    ])


def _layout_b(cfg):
    NT, KLO, KHI = cfg["NT"], cfg["KLO"], cfg["KHI"]
    K = KLO + KHI
    return _mk_layout([
        ("idx_lo", (NT, 16, KLO * 8), I16),
        ("idx_hi", (NT, 16, max(KHI * 8, 1)), I16),
        ("dlocr", (NT, K * P), I8),
    ])


def _prep_a(x, W1, att_src1, att_dst1, W2):
    """x shard + weights blob; independent of the edge structure."""
    x = np.asarray(x)
    N = x.shape[0]
    NT = -(-(-(-N // N_CORES)) // P)
    NPC = NT * P               # 128-aligned partition; core 7 is ragged
    assert (N_CORES - 1) * NPC < N <= N_CORES * NPC, N
    lay, tot16 = _layout_a(NPC, NT)
    blob = np.zeros((N_CORES, tot16), np.int16)

    xf = np.asarray(x, dtype=np.float32)
    rowmax = np.maximum(xf.max(axis=1), -xf.min(axis=1))
    pos_m = rowmax > 0
    inv = np.where(pos_m, 127.0 / np.where(pos_m, rowmax, 1.0), 0.0).astype(
        np.float32
    )
    y = xf * inv[:, None]
    np.rint(y, out=y)
    scale = (np.where(pos_m, rowmax, 0.0) / 127.0).astype(np.float32)
    CH = (N_CORES - 1) * NPC   # cores 0..6 are full; core 7 is ragged
    off, shp, _ = lay["xq"]
    blob_i8 = blob.view(np.int8)
    # y holds exact integers after rint, so the unsafe cast is exact
    np.copyto(
        blob_i8[:-1, 2 * off : 2 * off + NPC * F_IN].reshape(N_CORES - 1, NPC, F_IN),
        y[:CH].reshape(N_CORES - 1, NPC, F_IN), casting="unsafe",
    )
    np.copyto(
        blob_i8[-1, 2 * off : 2 * off + (N - CH) * F_IN].reshape(N - CH, F_IN),
        y[CH:], casting="unsafe",
    )
    off, shp, _ = lay["xscale"]
    blob[:-1, off : off + NPC * 2] = (
        scale[:CH].reshape(N_CORES - 1, -1).view(np.int16)
    )
    blob[-1, off : off + (N - CH) * 2] = scale[CH:].view(np.int16)

    W1 = np.asarray(W1, dtype=np.float32)
    Ws = np.stack(
        [W1[:, h * C1 : (h + 1) * C1] @ np.asarray(att_src1)[h] for h in range(H1)],
        axis=1,
    )
    Wd = np.stack(
        [W1[:, h * C1 : (h + 1) * C1] @ np.asarray(att_dst1)[h] for h in range(H1)],
        axis=1,
    )
    W1ext = np.concatenate([W1, Ws, Wd], axis=1).astype(ml_dtypes.bfloat16)

    def put1(name, arr, dt_):
        off, shp, _ = lay[name]
        arr = np.ascontiguousarray(arr, dtype=_NPD[dt_])
        flat = arr.view(np.int16).reshape(-1)
        blob[:, off : off + flat.size] = flat[None, :]

    put1("W1ext", W1ext, BF16)
    put1("iota1", np.arange(P, dtype=np.float32).reshape(1, P), BF16)
    put1("W2row", np.asarray(W2, dtype=np.float32).reshape(1, D1), F32)
    put1("iotac", np.arange(P, dtype=np.float32).reshape(P, 1), F32)
    return blob


def _prep_b(edge_index, N):
    """Edge tables blob + the data-dependent part of cfg."""
    NT = -(-(-(-N // N_CORES)) // P)
    NPC = NT * P               # 128-aligned partition; core 7 is ragged
    NG = N_CORES * NT

    ei = np.asarray(edge_index)
    E = ei.shape[1]
    ET = E + N
    src = np.empty(ET, np.int32)
    src[:E] = ei[0]
    src[E:] = np.arange(N, dtype=np.int32)
    dst = np.empty(ET, np.int32)
    dst[:E] = ei[1]
    dst[E:] = np.arange(N, dtype=np.int32)

    # NPC is a multiple of 128, so global 128-tiles and per-core tiles
    # coincide: group and local row come from shifts alone
    dloc = dst & 127
    hi = src >= SPLIT_AT
    # pre-masked int16 sources: == src - SPLIT_AT for hi, == src for lo
    src16 = (src & (SPLIT_AT - 1)).astype(np.int16)
    key = (dst >> 7).astype(np.int16)  # grp < NG << 32768; radix sorts fast
    key <<= 1
    key |= hi

    kcnt = np.bincount(key, minlength=2 * NG)
    cnt_lo, cnt_hi = kcnt[0::2], kcnt[1::2]
    KLO = int(-(-max(1, int(cnt_lo.max())) // P))
    KHI = int(-(-int(cnt_hi.max()) // P)) if cnt_hi.max() > 0 else 0
    K = KLO + KHI

    order = np.argsort(key, kind="stable")
    key_o = key[order]
    src_o16, dloc_o = src16[order], dloc[order]
    grp_o = key_o >> 1   # int16 is fine as a fancy index
    kstart = np.zeros(2 * NG, np.int32)
    np.cumsum(kcnt[:-1].astype(np.int32), out=kstart[1:])
    # fold the hi-table offset into the start table: odd keys (hi) place
    # their run at column KLO*P, so slot is one subtraction
    kstart[1::2] -= KLO * P
    slot = np.arange(ET, dtype=np.int32) - kstart[key_o]

    # dst-local index per slot, [K, P] (k-major row = dlocr layout),
    # sentinel -1 -> one-hot never matches -> padded slots drop out;
    # ships as raw int8, converted to bf16 on device (exact for -1..127)
    dloc_kp = np.full((NG, K, P), -1, dtype=np.int8)
    dloc_kp[grp_o, slot >> 7, slot & 127] = dloc_o
    dlocr = dloc_kp.reshape(NG, K * P)

    # one scatter for both tables (lo runs at columns [0, KLO*P), hi
    # runs at [KLO*P, K*P)); pads gather row 0
    dense = np.zeros((NG, K * P), np.int16)
    dense[grp_o, slot] = src_o16

    cfg = dict(N=N, NPC=NPC, NT=NT, KLO=KLO, KHI=KHI)
    lay, tot16 = _layout_b(cfg)
    blob = np.zeros((N_CORES, tot16), np.int16)
    NTC = NG // N_CORES

    def put_wrapped(name, cols, KK):
        # [NG, KK*P] slice -> blob as [NG, 16, KK*8] (position i ->
        # [i%16, i//16]; 8x Q7 replication happens on device), written
        # directly into the blob view to skip an intermediate copy
        off, shp, dt_ = lay[name]
        n16 = int(np.prod(shp)) * mybir.dt.size(dt_) // 2
        view = blob[:, off : off + n16].reshape(N_CORES, NTC, 16, KK * 8)
        view[:] = cols.reshape(N_CORES, NTC, KK * 8, 16).transpose(0, 1, 3, 2)

    put_wrapped("idx_lo", dense[:, : KLO * P], KLO)
    if KHI:
        put_wrapped("idx_hi", dense[:, KLO * P :], KHI)
    off, shp, dt_ = lay["dlocr"]
    blob[:, off : off + NTC * K * P // 2] = (
        dlocr.reshape(N_CORES, -1).view(np.int16)
    )
    return cfg, blob


def _build_program(cfg):
    import os as _os
    phases = int(_os.environ.get("GAT_PHASES", "5"))  # bisection aid
    N, NPC, NT = cfg["N"], cfg["NPC"], cfg["NT"]
    KLO, KHI = cfg["KLO"], cfg["KHI"]
    s2, d2 = cfg["s2"], cfg["d2"]
    K = KLO + KHI
    N2 = N_CORES * NPC         # padded node-table rows (node id == row id)
    NLO = min(N2, SPLIT_AT)
    lay_a, tot_a = _layout_a(NPC, NT)
    lay_b, tot_b = _layout_b(cfg)

    nc = bacc.Bacc("TRN2", target_bir_lowering=False, debug=False,
                   num_devices=N_CORES)

    blob_a = nc.dram_tensor("blob_a", [tot_a], I16, kind="ExternalInput")
    blob_b = nc.dram_tensor("blob_b", [tot_b], I16, kind="ExternalInput")
    out = nc.dram_tensor("out", [NPC, 1], F32, kind="ExternalOutput")

    def sect(blob_d, lay, name):
        off, shp, dt_ = lay[name]
        n16 = int(np.prod(shp)) * mybir.dt.size(dt_) // 2
        ap = blob_d[off : off + n16]
        if dt_ != I16:
            ap = ap.bitcast(dt_)
        if len(shp) == 2:
            ap = ap.rearrange("(a b) -> a b", b=shp[1])
        else:
            ap = ap.rearrange("(a b c) -> a b c", b=shp[1], c=shp[2])
        return ap

    xq_ap = sect(blob_a, lay_a, "xq")
    xsc_ap = sect(blob_a, lay_a, "xscale")
    w1_ap = sect(blob_a, lay_a, "W1ext")
    iota1_ap = sect(blob_a, lay_a, "iota1")
    w2_ap = sect(blob_a, lay_a, "W2row")
    iotac_ap = sect(blob_a, lay_a, "iotac")
    idxlo_ap = sect(blob_b, lay_b, "idx_lo")
    idxhi_ap = sect(blob_b, lay_b, "idx_hi")
    dlocr_ap = sect(blob_b, lay_b, "dlocr")

    with tile.TileContext(nc) as tc:
        with (
            tc.tile_pool(name="dram", bufs=1, space="DRAM") as dram,
            tc.tile_pool(name="const", bufs=1) as constp,
            tc.tile_pool(name="p1", bufs=4) as p1,
            tc.tile_pool(name="p1ps", bufs=1, space="PSUM") as p1ps,
            tc.tile_pool(name="meta", bufs=3) as metap,
            tc.tile_pool(name="gath", bufs=3) as gathp,
            tc.tile_pool(name="work", bufs=2) as workp,
            tc.tile_pool(name="spool", bufs=4) as spool,
            tc.tile_pool(name="ps_out", bufs=2, space="PSUM") as ps_out,
            tc.tile_pool(name="ps_ad", bufs=2, space="PSUM") as ps_ad,
            tc.tile_pool(name="ps_bc", bufs=2, space="PSUM") as ps_bc,
        ):
            Rtab_shard = dram.tile([NPC, RECW], BF16)
            Rtab = dram.tile([N2, RECW], BF16)
            r2_shard = dram.tile([NPC, L2W], F32)
            r2_full = dram.tile([N2, L2W], F32)

            w1_sb = constp.tile([F_IN, D1 + 2 * H1], BF16)
            nc.sync.dma_start(out=w1_sb[:], in_=w1_ap)
            iotac_sb = constp.tile([P, 1], F32)
            nc.sync.dma_start(out=iotac_sb[:], in_=iotac_ap)
            ones_sb = constp.tile([1, P], BF16)
            nc.vector.memset(ones_sb[:], 1.0)
            # replicate single rows across partitions by doubling
            iota_sb = constp.tile([P, P], BF16)
            nc.sync.dma_start(out=iota_sb[0:1, :], in_=iota1_ap)
            w2_sb = constp.tile([P, D1], F32)
            nc.sync.dma_start(out=w2_sb[0:1, :], in_=w2_ap)
            r = 1
            while r < P:
                nc.sync.dma_start(out=iota_sb[r : 2 * r, :], in_=iota_sb[0:r, :])
                nc.sync.dma_start(out=w2_sb[r : 2 * r, :], in_=w2_sb[0:r, :])
                r *= 2
            # identity for the fused dequant+transpose matmul
            ident = constp.tile([P, P], BF16)
            nc.vector.tensor_scalar(
                out=ident[:], in0=iota_sb[:], scalar1=iotac_sb[:],
                scalar2=None, op0=mybir.AluOpType.is_equal,
            )
            adbuf = constp.tile([P, NT * H1], BF16)   # a_d of local dst tiles
            nc.vector.memset(adbuf[:], 0.0)
            x2buf = constp.tile([P, NT], F32)         # xp2 of local dst tiles
            nc.vector.memset(x2buf[:], 0.0)

            # NaN-proof gather destinations once (skipped -1 slots keep stale
            # SBUF contents), and the record staging tiles' pad columns.
            for _ in range(3):
                z1 = gathp.tile([P, K * RECW], BF16, tag="gr")
                nc.vector.memset(z1[:], 0.0)
                z2 = gathp.tile([P, K * L2W], F32, tag="gr2")
                nc.vector.memset(z2[:], 0.0)

            # ---------------- phase 1: local node precompute ----------------
            for t in range(NT if phases >= 1 else 0):
                n0 = t * P
                w = min(P, NPC - n0)
                # int8 x tile [node, f] -> bf16, then one matmul against a
                # per-node diagonal scale both dequantizes and transposes:
                # psT[f, n] = sum_p xqb[p, f] * (scale[p] * (p == n))
                xq8 = p1.tile([P, F_IN], I8, tag="xq8")
                nc.sync.dma_start(out=xq8[:], in_=xq_ap[n0 : n0 + P, :])
                xqb = p1.tile([P, F_IN], BF16, tag="xqb")
                nc.scalar.copy(out=xqb[:], in_=xq8[:])
                sc = p1.tile([P, 1], F32, tag="sc")
                nc.sync.dma_start(out=sc[:], in_=xsc_ap[n0 : n0 + P, :])
                sdiag = p1.tile([P, P], BF16, tag="sdiag")
                nc.vector.tensor_scalar(
                    out=sdiag[:], in0=ident[:], scalar1=sc[:], scalar2=None,
                    op0=mybir.AluOpType.mult,
                )
                psT = p1ps.tile([P, P], F32, tag="psT")
                nc.tensor.matmul(
                    out=psT[:], lhsT=xqb[:], rhs=sdiag[:], start=True,
                    stop=True,
                )
                xt = p1.tile([F_IN, P], BF16, tag="xt")
                nc.scalar.copy(out=xt[:], in_=psT[:])
                ps = p1ps.tile([P, D1 + 2 * H1], F32, tag="p1ps")
                nc.tensor.matmul(
                    out=ps[:], lhsT=xt[:], rhs=w1_sb[:], start=True, stop=True,
                )
                rec = p1.tile([P, RECW], BF16, tag="rec")
                nc.vector.memset(rec[:, D1 + H1 :], 0.0)
                nc.vector.tensor_copy(
                    out=rec[:, : D1 + H1], in_=ps[:, : D1 + H1]
                )
                nc.sync.dma_start(out=Rtab_shard[n0 : n0 + w, :], in_=rec[:w, :])
                nc.scalar.copy(
                    out=adbuf[:w, t * H1 : (t + 1) * H1],
                    in_=ps[:w, D1 + H1 : D1 + 2 * H1],
                )

            # share the record table: shard -> full (rows land at global ids)
            if phases >= 2:
                nc.gpsimd.collective_compute(
                    "AllGather",
                    mybir.AluOpType.bypass,
                    replica_groups=[list(range(N_CORES))],
                    ins=[Rtab_shard[:].opt()],
                    outs=[Rtab[:].opt()],
                )

            def load_idx(ap_t, cols, tag):
                """[16, cols] DRAM -> [128, cols] SBUF, replicating across
                the 8 Q7 partition groups by doubling."""
                ix = metap.tile([P, cols], I16, tag=tag)
                nc.sync.dma_start(out=ix[0:16, :], in_=ap_t)
                nc.sync.dma_start(out=ix[16:32, :], in_=ix[0:16, :])
                nc.sync.dma_start(out=ix[32:64, :], in_=ix[0:32, :])
                nc.sync.dma_start(out=ix[64:128, :], in_=ix[0:64, :])
                return ix

            # ---------------- phase 2: layer-1 edges ------------------------
            for t in range(NT if phases >= 3 else 0):
                n0 = t * P
                w = min(P, NPC - n0)
                ilo = load_idx(idxlo_ap[t], max(KLO * 8, 1), "ilo")
                m_dlr8 = metap.tile([1, K * P], I8, tag="mdlr8")
                nc.sync.dma_start(out=m_dlr8[:], in_=dlocr_ap[t : t + 1, :])
                m_dlr = metap.tile([1, K * P], BF16, tag="mdlr")
                nc.scalar.copy(out=m_dlr[:], in_=m_dlr8[:])

                gr = gathp.tile([P, K * RECW], BF16, tag="gr")
                gr3 = gr[:].rearrange("p (k c) -> p k c", c=RECW)
                nc.gpsimd.dma_gather(
                    out_ap=gr3[:, :KLO, :], in_ap=Rtab[:][:NLO, :],
                    idxs_ap=ilo[:], num_idxs=KLO * P, num_idxs_reg=KLO * P,
                    elem_size=RECW, single_packet=False,
                )
                if KHI:
                    ihi = load_idx(idxhi_ap[t], KHI * 8, "ihi")
                    nc.gpsimd.dma_gather(
                        out_ap=gr3[:, KLO:, :], in_ap=Rtab[:][SPLIT_AT:, :],
                        idxs_ap=ihi[:], num_idxs=KHI * P, num_idxs_reg=KHI * P,
                        elem_size=RECW, single_packet=False,
                    )

                # ST_all[j, k*128+e] = (dlocr[k*128+e] == j)
                st_all = spool.tile([P, K * P], BF16, tag="st_all")
                for c0 in range(0, K * P, 512):
                    cw = min(512, K * P - c0)
                    psb = ps_bc.tile([P, 512], F32, tag="psb")
                    nc.tensor.matmul(
                        out=psb[:, :cw], lhsT=ones_sb[:],
                        rhs=m_dlr[:, c0 : c0 + cw], start=True, stop=True,
                    )
                    nc.vector.tensor_scalar(
                        out=st_all[:, c0 : c0 + cw], in0=psb[:, :cw],
                        scalar1=iotac_sb[:], scalar2=None,
                        op0=mybir.AluOpType.is_equal,
                    )

                # a_d expansion: psum[e, k*8+h] = ST_k.T @ adbuf[:, tile t]
                ps_adw = ps_ad.tile([P, K * H1], F32, tag="ps_adw")
                for k in range(K):
                    nc.tensor.matmul(
                        out=ps_adw[:, k * H1 : (k + 1) * H1],
                        lhsT=st_all[:, k * P : (k + 1) * P],
                        rhs=adbuf[:, t * H1 : (t + 1) * H1],
                        start=True, stop=True,
                    )

                lg = workp.tile([P, K * H1], F32, tag="lg")
                nc.vector.tensor_add(
                    out=lg[:].rearrange("p (k h) -> p k h", h=H1),
                    in0=gr3[:, :, D1 : D1 + H1],
                    in1=ps_adw[:].rearrange("p (k h) -> p k h", h=H1),
                )
                nc.vector.scalar_tensor_tensor(
                    out=lg[:], in0=lg[:], scalar=NEG, in1=lg[:],
                    op0=mybir.AluOpType.mult, op1=mybir.AluOpType.max,
                )
                exb = workp.tile([P, K * H1], BF16, tag="exb")
                nc.scalar.activation(out=exb[:], in_=lg[:], func=AF.Exp)

                rhs = gathp.tile([P, K * (H1 + D1)], BF16, tag="rhs")
                rhs3 = rhs[:].rearrange("p (k c) -> p k c", c=H1 + D1)
                exb3 = exb[:].rearrange("p (k h) -> p k h", h=H1)
                nc.vector.tensor_copy(out=rhs3[:, :, 0:H1], in_=exb3[:])
                ex4 = exb3[:, :, :, None].to_broadcast([P, K, H1, C1])
                nc.vector.tensor_mul(
                    out=rhs3[:, :, H1:].rearrange("p k (h c) -> p k h c", c=C1),
                    in0=gr3[:, :, 0:D1].rearrange("p k (h c) -> p k h c", c=C1),
                    in1=ex4,
                )

                pso = ps_out.tile([P, H1 + D1], F32, tag="pso")
                for k in range(K):
                    # s_sb (edge-major one-hot) = transpose of st_all chunk
                    psb2 = ps_bc.tile([P, 512], F32, tag="psb")
                    nc.tensor.matmul(
                        out=psb2[:, :P], lhsT=st_all[:, k * P : (k + 1) * P],
                        rhs=ident[:], start=True, stop=True,
                    )
                    s_sb = spool.tile([P, P], BF16, tag="s_sb")
                    nc.vector.tensor_copy(out=s_sb[:], in_=psb2[:, :P])
                    nc.tensor.matmul(
                        out=pso[:], lhsT=s_sb[:], rhs=rhs3[:, k, :],
                        start=(k == 0), stop=(k == K - 1),
                    )

                rec_t = workp.tile([P, H1], F32, tag="rec_t")
                nc.vector.tensor_scalar_max(
                    out=rec_t[:], in0=pso[:, 0:H1], scalar1=1e-30
                )
                nc.vector.reciprocal(out=rec_t[:], in_=rec_t[:])
                h1 = workp.tile([P, D1], F32, tag="h1")
                r4 = rec_t[:][:, :, None].to_broadcast([P, H1, C1])
                nc.vector.tensor_mul(
                    out=h1[:].rearrange("p (h c) -> p h c", c=C1),
                    in0=pso[:, H1:].rearrange("p (h c) -> p h c", c=C1),
                    in1=r4,
                )
                tmin = workp.tile([P, D1], F32, tag="tmin")
                nc.vector.tensor_scalar_min(out=tmin[:], in0=h1[:], scalar1=0.0)
                nc.scalar.activation(out=tmin[:], in_=tmin[:], func=AF.Exp)
                trelu = workp.tile([P, D1], F32, tag="trelu")
                nc.scalar.activation(out=trelu[:], in_=h1[:], func=AF.Relu)
                nc.vector.tensor_add(out=h1[:], in0=trelu[:], in1=tmin[:])
                nc.vector.tensor_scalar_add(out=h1[:], in0=h1[:], scalar1=-1.0)

                m2 = workp.tile([P, D1], F32, tag="m2")
                nc.vector.tensor_mul(out=m2[:], in0=h1[:], in1=w2_sb[:])
                xp2c = workp.tile([P, L2W], F32, tag="xp2c")
                nc.vector.memset(xp2c[:], 0.0)
                nc.vector.tensor_reduce(
                    out=xp2c[:, 0:1], in_=m2[:], axis=mybir.AxisListType.X,
                    op=mybir.AluOpType.add,
                )
                nc.scalar.copy(out=x2buf[:, t : t + 1], in_=xp2c[:, 0:1])
                nc.sync.dma_start(out=r2_shard[n0 : n0 + w, :], in_=xp2c[:w, :])

            # ---------------- all-gather layer-2 node scalars ---------------
            if phases >= 4:
                nc.gpsimd.collective_compute(
                    "AllGather",
                    mybir.AluOpType.bypass,
                    replica_groups=[list(range(N_CORES))],
                    ins=[r2_shard[:].opt()],
                    outs=[r2_full[:].opt()],
                )

            # ---------------- phase 3: layer-2 edges ------------------------
            for t in range(NT if phases >= 5 else 0):
                n0 = t * P
                w = min(P, NPC - n0)
                ilo = load_idx(idxlo_ap[t], max(KLO * 8, 1), "ilo")
                m_dlr8 = metap.tile([1, K * P], I8, tag="mdlr8")
                nc.sync.dma_start(out=m_dlr8[:], in_=dlocr_ap[t : t + 1, :])
                m_dlr = metap.tile([1, K * P], BF16, tag="mdlr")
                nc.scalar.copy(out=m_dlr[:], in_=m_dlr8[:])

                gr2 = gathp.tile([P, K * L2W], F32, tag="gr2")
                g23 = gr2[:].rearrange("p (k c) -> p k c", c=L2W)
                nc.gpsimd.dma_gather(
                    out_ap=g23[:, :KLO, :], in_ap=r2_full[:][:NLO, :],
                    idxs_ap=ilo[:], num_idxs=KLO * P, num_idxs_reg=KLO * P,
                    elem_size=L2W, single_packet=False,
                )
                if KHI:
                    ihi = load_idx(idxhi_ap[t], KHI * 8, "ihi")
                    nc.gpsimd.dma_gather(
                        out_ap=g23[:, KLO:, :], in_ap=r2_full[:][SPLIT_AT:, :],
                        idxs_ap=ihi[:], num_idxs=KHI * P, num_idxs_reg=KHI * P,
                        elem_size=L2W, single_packet=False,
                    )

                st_all = spool.tile([P, K * P], BF16, tag="st_all")
                for c0 in range(0, K * P, 512):
                    cw = min(512, K * P - c0)
                    psb = ps_bc.tile([P, 512], F32, tag="psb")
                    nc.tensor.matmul(
                        out=psb[:, :cw], lhsT=ones_sb[:],
                        rhs=m_dlr[:, c0 : c0 + cw], start=True, stop=True,
                    )
                    nc.vector.tensor_scalar(
                        out=st_all[:, c0 : c0 + cw], in0=psb[:, :cw],
                        scalar1=iotac_sb[:], scalar2=None,
                        op0=mybir.AluOpType.is_equal,
                    )
                x2wb = workp.tile([P, 1], BF16, tag="x2wb")
                nc.vector.tensor_copy(out=x2wb[:], in_=x2buf[:, t : t + 1])
                ps_xd = ps_ad.tile([P, K], F32, tag="ps_adw")
                for k in range(K):
                    nc.tensor.matmul(
                        out=ps_xd[:, k : k + 1],
                        lhsT=st_all[:, k * P : (k + 1) * P],
                        rhs=x2wb[:], start=True, stop=True,
                    )

                gs = g23[:, :, 0]  # [P, K] xp2[src]
                lg2 = workp.tile([P, K], F32, tag="lg2")
                nc.vector.tensor_scalar_mul(out=lg2[:], in0=ps_xd[:], scalar1=d2)
                nc.vector.scalar_tensor_tensor(
                    out=lg2[:], in0=gs, scalar=s2, in1=lg2[:],
                    op0=mybir.AluOpType.mult, op1=mybir.AluOpType.add,
                )
                nc.vector.scalar_tensor_tensor(
                    out=lg2[:], in0=lg2[:], scalar=NEG, in1=lg2[:],
                    op0=mybir.AluOpType.mult, op1=mybir.AluOpType.max,
                )
                ex2 = workp.tile([P, K], BF16, tag="ex2")
                nc.scalar.activation(out=ex2[:], in_=lg2[:], func=AF.Exp)
                rhs2 = workp.tile([P, K * 2], BF16, tag="rhs2")
                rhs2v = rhs2[:].rearrange("p (k c) -> p k c", c=2)
                nc.vector.tensor_copy(out=rhs2v[:, :, 0:1], in_=ex2[:, :, None])
                nc.vector.tensor_mul(
                    out=rhs2v[:, :, 1:2], in0=ex2[:, :, None], in1=gs[:, :, None]
                )

                pso2 = ps_out.tile([P, 2], F32, tag="pso")
                for k in range(K):
                    psb2 = ps_bc.tile([P, 512], F32, tag="psb")
                    nc.tensor.matmul(
                        out=psb2[:, :P], lhsT=st_all[:, k * P : (k + 1) * P],
                        rhs=ident[:], start=True, stop=True,
                    )
                    s_sb = spool.tile([P, P], BF16, tag="s_sb")
                    nc.vector.tensor_copy(out=s_sb[:], in_=psb2[:, :P])
                    nc.tensor.matmul(
                        out=pso2[:], lhsT=s_sb[:], rhs=rhs2v[:, k, :],
                        start=(k == 0), stop=(k == K - 1),
                    )

                rec2 = workp.tile([P, 1], F32, tag="rec2")
                nc.vector.tensor_scalar_max(
                    out=rec2[:], in0=pso2[:, 0:1], scalar1=1e-30
                )
                nc.vector.reciprocal(out=rec2[:], in_=rec2[:])
                o_t = workp.tile([P, 1], F32, tag="o_t")
                nc.vector.tensor_mul(out=o_t[:], in0=pso2[:, 1:2], in1=rec2[:])
                nc.sync.dma_start(out=out[n0 : n0 + w, :], in_=o_t[:w, :])

    return nc


# compiled program + jitted executable, keyed on everything baked into
# the program (shapes and the scalar attention weights)
_CACHE: dict = {}
_SHD = None


def _get_shd():
    global _SHD
    if _SHD is None:
        import jax
        from jax.sharding import Mesh, PartitionSpec, NamedSharding
        devices = jax.devices()[:N_CORES]
        mesh = Mesh(np.asarray(devices), ("core",))
        _SHD = NamedSharding(mesh, PartitionSpec("core"))
    return _SHD


def _make_runner(nc):
    import jax
    import concourse.mybir as mb
    from jax.experimental.shard_map import shard_map
    from jax.sharding import PartitionSpec
    from concourse import bass2jax as b2j

    b2j.install_neuronx_cc_hook()
    partition_name = nc.partition_id_tensor.name if nc.partition_id_tensor else None
    in_names, out_names, out_avals, zero_outs = [], [], [], []
    for alloc in nc.m.functions[0].allocations:
        if not isinstance(alloc, mb.MemoryLocationSet):
            continue
        name = alloc.memorylocations[0].name
        if alloc.kind == "ExternalInput":
            if name != partition_name:
                in_names.append(name)
        elif alloc.kind == "ExternalOutput":
            shape = tuple(alloc.tensor_shape)
            dtype = mb.dt.np(alloc.dtype)
            out_names.append(name)
            out_avals.append(jax.core.ShapedArray(shape, dtype))
            zero_outs.append(np.zeros(shape, dtype))
    assert in_names == ["blob_a", "blob_b"], in_names
    n_params = len(in_names)
    n_outs = len(out_avals)
    all_in_names = list(in_names) + list(out_names)
    if partition_name is not None:
        all_in_names.append(partition_name)

    def _body(*args):
        operands = list(args)
        if partition_name is not None:
            operands.append(b2j.partition_id_tensor())
        return tuple(
            b2j._bass_exec_p.bind(
                *operands, out_avals=tuple(out_avals),
                in_names=tuple(all_in_names), out_names=tuple(out_names),
                lowering_input_output_aliases=(), sim_require_finite=True,
                sim_require_nnan=True, nc=nc,
            )
        )

    shd = _get_shd()
    mesh = shd.mesh
    spec = shd.spec
    in_specs = (spec,) * (n_params + n_outs)
    out_specs = (spec,) * n_outs
    sharded = jax.jit(
        shard_map(_body, mesh=mesh, in_specs=in_specs, out_specs=out_specs,
                  check_rep=False),
        keep_unused=True,
    )
    # outputs are separate custom-call results (no aliasing declared), so
    # the zero-filled operands can be allocated once and reused per call
    zeros_dev = [
        jax.device_put(np.zeros((N_CORES * z.shape[0], *z.shape[1:]), z.dtype),
                       shd)
        for z in zero_outs
    ]

    def run(bd_a, bd_b):
        outs = sharded(bd_a, bd_b, *zeros_dev)
        return np.asarray(outs[0])

    return run


def kernel(x, edge_index, W1, att_src1, att_dst1, b1, W2, att_src2, att_dst2, b2):
    import jax
    assert not np.any(np.asarray(b1)) and not np.any(np.asarray(b2)), (
        "bias folding not implemented (biases are zero for this problem)"
    )
    shd = _get_shd()
    N = np.asarray(x).shape[0]

    # single host core: threading cannot parallelize the numpy work, it
    # only delays the blob-A dispatch. Run prep_a clean, dispatch A as
    # early as possible (its wire time then overlaps the edge prep), and
    # let prep_b contend only with A's background serialization.
    blob_a = _prep_a(x, W1, att_src1, att_dst1, W2)
    bd_a = jax.device_put(blob_a.reshape(-1), shd)
    cfg, blob_b = _prep_b(edge_index, N)
    cfg["s2"] = float(np.asarray(att_src2).reshape(-1)[0])
    cfg["d2"] = float(np.asarray(att_dst2).reshape(-1)[0])
    bd_b = jax.device_put(blob_b.reshape(-1), shd)

    key = (cfg["N"], cfg["NPC"], cfg["NT"], cfg["KLO"], cfg["KHI"],
           cfg["s2"], cfg["d2"])
    run = _CACHE.get(key)
    if run is None:
        nc = _build_program(cfg)
        nc.compile()
        _split_sync_waits(nc)
        run = _make_runner(nc)
        _CACHE[key] = run
    # rows are node ids in the padded per-core layout; cores 0..6 are
    # full so the first N rows are exactly nodes 0..N-1
    return run(bd_a, bd_b)[:N]
